# revision 24
# baseline (speedup 1.0000x reference)
"""Trainium2 Bass kernel for nn_Attention_layer (dense_transformer).

One batch element per NeuronCore (8 cores). Positions are hw-major:
pos = hw*64 + d, so per-hw [*, 64] slices and out-proj chunks are contiguous.

Host precompute folds all biases into exact linear algebra:
  x' = x + bo          (residual carries bo; projections corrected below)
  k = wk x' + (bk - wk bo)      q = wq x' + (bq - wq bo)
  vT'[pos,s] = (x'^T wv^T)[pos,s] = (wv x)[s,pos] + (wv bo)[s]   (no bias op)
  att' = vT'^T a = att_u + wv bo   (softmax cols sum to 1)
  att  = att' + (bv - wv bo)       (bias at att eviction)
  out  = wo att + x'               (single residual add, = reference exactly)

PSUM->SBUF eviction throughput (~1ns/elem/engine on Act+DVE, big fixed cost
per op) is the limiting resource after DMA, so evictions are merged into
[128,1024] two-bank tiles via a single unified 4-slot PSUM pool.

Per batch of 32 hw (2048 pos), software-pipelined across batches:
  PE: scores(b) -> proj k/q/vT'(b+1) -> aT->a transposes(b) -> att(b) -> out(b)
  Act: exp, q/att(+bias)/tr evictions, half the residual adds
  DVE: denom reduce+rcp, k/vT evictions, half the residual adds
  Pool(gpsimd): aT normalization (SBUF only)
  SP: all DMA (x loads, bf16 out stores; large contiguous transfers)
"""

import numpy as np
import ml_dtypes

import concourse.bacc as bacc
import concourse.tile as tile
from concourse import mybir
from concourse.bass_utils import run_bass_kernel_spmd

F32 = mybir.dt.float32
BF16 = mybir.dt.bfloat16
AF = mybir.ActivationFunctionType

B, C, S, D, H, W = 8, 256, 128, 64, 16, 16
HW = H * W              # 256
NPOS = HW * D           # 16384, pos = hw*64 + d
NBATCH = 8              # batches of 32 hw
BPOS = NPOS // NBATCH   # 2048 pos per batch
SCALE = float(1.0 / np.sqrt(np.float32(S)))

CFG = {
    "loop_n": 1,      # on-device repeats of the whole body (timing)
    "stage": "full",  # debug: dma | proj | scores | tr | att | full
}

_CACHE = {}


def _emit(nc, tc, io, ctx):
    xb, wkT, wqT, wvT, woT, bk2, bq2, catt, ident, out_d = io

    # ---- pools ----------------------------------------------------------
    const = ctx.enter_context(tc.tile_pool(name="const", bufs=1))
    xpool = ctx.enter_context(tc.tile_pool(name="xpool", bufs=1))
    kqp = ctx.enter_context(tc.tile_pool(name="kqp", bufs=2))
    vp = ctx.enter_context(tc.tile_pool(name="vp", bufs=2))
    ap = ctx.enter_context(tc.tile_pool(name="ap", bufs=2))
    attp = ctx.enter_context(tc.tile_pool(name="attp", bufs=2))
    op = ctx.enter_context(tc.tile_pool(name="op", bufs=2))
    # unified PSUM pool: 4 slots x [128,1024] f32 (2 banks each)
    pu = ctx.enter_context(tc.tile_pool(name="pu", bufs=4, space="PSUM"))

    # ---- constants ------------------------------------------------------
    id_sb = const.tile([128, 128], BF16, tag="ident")
    nc.sync.dma_start(id_sb[:], ident[:])
    wk_sb, wq_sb, wv_sb = {}, {}, {}
    for h in range(2):
        sl = slice(h * 128, (h + 1) * 128)
        wk_sb[h] = const.tile([128, 128], BF16, tag=f"wk{h}", name=f"wk{h}")
        nc.sync.dma_start(wk_sb[h][:], wkT[sl, :])
        wq_sb[h] = const.tile([128, 128], BF16, tag=f"wq{h}", name=f"wq{h}")
        nc.sync.dma_start(wq_sb[h][:], wqT[sl, :])
        wv_sb[h] = const.tile([128, 128], BF16, tag=f"wv{h}", name=f"wv{h}")
        nc.sync.dma_start(wv_sb[h][:], wvT[sl, :])
    wo_sb = const.tile([128, 256], BF16, tag="wo")
    nc.sync.dma_start(wo_sb[:], woT[:])
    bk_sb = const.tile([128, 1], F32, tag="bk")
    nc.sync.dma_start(bk_sb[:], bk2[:])
    bq_sb = const.tile([128, 1], F32, tag="bq")
    nc.sync.dma_start(bq_sb[:], bq2[:])
    ca_sb = const.tile([128, 1], F32, tag="ca")
    nc.sync.dma_start(ca_sb[:], catt[:])

    loop_cm = tc.For_i(0, CFG["loop_n"], 1) if CFG["loop_n"] > 1 else None
    if loop_cm is not None:
        ctx.enter_context(loop_cm)

    # ---- x load (16 contiguous 0.5 MB DMAs) -----------------------------
    x_sb = [xpool.tile([128, NPOS], BF16, tag=f"x{h}", name=f"x_sb{h}")
            for h in range(2)]
    for ch in range(8):
        sl = slice(ch * 2048, (ch + 1) * 2048)
        for h in range(2):
            nc.sync.dma_start(x_sb[h][:, sl], xb[h * 128:(h + 1) * 128, sl])

    state = {}
    stage = CFG["stage"]

    def dump(b, t, half=0):
        # debug: convert to f32 staging and store as the batch's out rows
        bsl = slice(b * BPOS, (b + 1) * BPOS)
        w = t.shape[1]
        o_t = op.tile([128, BPOS], BF16, tag=f"o{half}", name=f"dump{b}{half}")
        nc.vector.tensor_copy(o_t[:, 0:w], t[:])
        if w < BPOS:
            nc.vector.memset(o_t[:, w:BPOS], 0.0)
        nc.sync.dma_start(out_d[half * 128:(half + 1) * 128, bsl], o_t[:])

    def emit_proj(b):
        k_t = kqp.tile([128, BPOS], BF16, tag="k", name=f"k{b}")
        q_t = kqp.tile([128, BPOS], BF16, tag="q", name=f"q{b}")
        vT_t = vp.tile([128, BPOS], BF16, tag="vT", name=f"vT{b}")
        state[b] = (k_t, q_t, vT_t)
        for nm, wsb, dst, bias in (("k", wk_sb, k_t, bk_sb),
                                   ("q", wq_sb, q_t, bq_sb)):
            for t2 in range(2):     # [128,1024] tile = 2 chunks of 512
                ps = pu.tile([128, 1024], F32, tag="u", name=f"p{nm}{b}{t2}")
                for c2 in range(2):
                    csl = slice(b * BPOS + (t2 * 2 + c2) * 512,
                                b * BPOS + (t2 * 2 + c2 + 1) * 512)
                    psl = ps[:, c2 * 512:(c2 + 1) * 512]
                    # each 512-col half is its own PSUM bank: start/stop per bank
                    nc.tensor.matmul(psl, wsb[0][:], x_sb[0][:, csl],
                                     start=True, stop=False)
                    nc.tensor.matmul(psl, wsb[1][:], x_sb[1][:, csl],
                                     start=False, stop=True)
                dsl = dst[:, t2 * 1024:(t2 + 1) * 1024]
                if nm == "k" and t2 == 1:
                    nc.vector.tensor_scalar_add(dsl, ps[:], bias[:])
                else:
                    nc.scalar.activation(dsl, ps[:], AF.Identity,
                                         bias=bias[:], scale=1.0)
        # vT': stationary = x' pos-block (1 pair = 128 pos), streams wvT
        for t2 in range(2):
            ps = pu.tile([128, 1024], F32, tag="u", name=f"pv{b}{t2}")
            for u in range(8):
                p0 = (b * 16 + t2 * 8 + u) * 128
                for h in range(2):
                    nc.tensor.matmul(
                        ps[:, u * 128:(u + 1) * 128],
                        x_sb[h][:, p0:p0 + 128], wv_sb[h][:],
                        start=(u % 4 == 0 and h == 0),
                        stop=(u % 4 == 3 and h == 1))
            nc.scalar.copy(vT_t[:, t2 * 1024:(t2 + 1) * 1024], ps[:])

    def emit_scores(b):
        k_t, q_t, _ = state[b]
        aT_t = ap.tile([128, 1024], BF16, tag="aT", name=f"aT{b}")
        den = ap.tile([128, 16], F32, tag="den", name=f"den{b}")
        rcp = ap.tile([128, 16], F32, tag="rcp", name=f"rcp{b}")
        state[b] += (aT_t,)
        ps = pu.tile([128, 1024], F32, tag="u", name=f"pe{b}")
        for p in range(16):
            for u in range(2):
                hw = (b * 16 + p) * 2 + u
                csl = slice(hw * 64 - b * BPOS, hw * 64 - b * BPOS + 64)
                nc.tensor.matmul(ps[u * 64:(u + 1) * 64,
                                    p * 64:(p + 1) * 64],
                                 q_t[:, csl], k_t[:, csl],
                                 start=(p % 8 == 0), stop=(p % 8 == 7),
                                 skip_group_check=True)
        nc.scalar.activation(aT_t[:], ps[:], AF.Exp, scale=SCALE)
        nc.vector.reduce_sum(
            out=den[:],
            in_=aT_t[:].rearrange("p (i f) -> p i f", i=16),
            axis=mybir.AxisListType.X)
        nc.vector.reciprocal(rcp[:], den[:])
        for p in range(16):
            nc.gpsimd.tensor_scalar_mul(aT_t[:, p * 64:(p + 1) * 64],
                                        aT_t[:, p * 64:(p + 1) * 64],
                                        rcp[:, p:p + 1])

    def emit_tail(b):
        bsl = slice(b * BPOS, (b + 1) * BPOS)
        _, _, vT_t, aT_t = state.pop(b)
        a_t = ap.tile([128, 1024], BF16, tag="a", name=f"a{b}")
        # aT -> a transposes: [64,64] blocks, even pair-halves at rows 0:64,
        # odd at 64:128 (matching vT' block layout)
        ps = pu.tile([128, 2048], BF16, tag="u", name=f"pt{b}")
        for p in range(16):
            for u in range(2):
                r0 = u * 64
                nc.tensor.matmul(ps[r0:r0 + 64, p * 64:(p + 1) * 64],
                                 aT_t[r0:r0 + 64, p * 64:(p + 1) * 64],
                                 id_sb[r0:r0 + 64, r0:r0 + 64],
                                 is_transpose=True,
                                 start=(p == 0), stop=(p == 15),
                                 skip_group_check=True)
        nc.vector.tensor_copy(a_t[:], ps[:, 0:1024])
        if stage == "tr":
            dump(b, a_t, 0)
            return
        # att: per hw, lhsT = vT' [64(i),128(s)], rhs = a [64(i),64(j)]
        att_t = attp.tile([128, BPOS], BF16, tag="att", name=f"att{b}")
        for t2 in range(2):
            ps = pu.tile([128, 1024], F32, tag="u", name=f"pa{b}{t2}")
            for u in range(16):
                p = (t2 * 16 + u) // 2
                osl = slice(u * 64, (u + 1) * 64)
                if u % 2 == 0:
                    nc.tensor.matmul(
                        ps[:, osl],
                        vT_t[0:64, p * 128:(p + 1) * 128],
                        a_t[0:64, p * 64:(p + 1) * 64],
                        start=(u % 8 == 0), stop=False, skip_group_check=True)
                else:
                    # odd hw: (64,0) with 128-wide out is illegal on HW;
                    # split into (64,0) and (64,64) quadrants
                    for sh in range(2):
                        nc.tensor.matmul(
                            ps[sh * 64:(sh + 1) * 64, osl],
                            vT_t[64:128,
                                 p * 128 + sh * 64: p * 128 + sh * 64 + 64],
                            a_t[64:128, p * 64:(p + 1) * 64],
                            start=False, stop=(u % 8 == 7 and sh == 1),
                            skip_group_check=True)
            nc.scalar.activation(att_t[:, t2 * 1024:(t2 + 1) * 1024], ps[:],
                                 AF.Identity, bias=ca_sb[:], scale=1.0)
        if stage == "att":
            dump(b, att_t, 0)
            return
        # out-proj + residual; h-major for stationary reuse
        for h in range(2):
            o_t = op.tile([128, BPOS], BF16, tag=f"o{h}", name=f"o{b}{h}")
            for t2 in range(2):
                ps = pu.tile([128, 1024], F32, tag="u", name=f"po{b}{h}{t2}")
                for c2 in range(2):
                    sl = slice((t2 * 2 + c2) * 512, (t2 * 2 + c2 + 1) * 512)
                    nc.tensor.matmul(ps[:, c2 * 512:(c2 + 1) * 512],
                                     wo_sb[:, h * 128:(h + 1) * 128],
                                     att_t[:, sl], start=True, stop=True,
                                     skip_group_check=True)
                osl = slice(t2 * 1024, (t2 + 1) * 1024)
                xa = x_sb[h][:, b * BPOS + t2 * 1024: b * BPOS + (t2 + 1) * 1024]
                nc.vector.tensor_add(o_t[:, osl], ps[:], xa)
            nc.sync.dma_start(out_d[h * 128:(h + 1) * 128, bsl], o_t[:])

    if stage == "dma":
        for b in range(NBATCH):
            bsl = slice(b * BPOS, (b + 1) * BPOS)
            for h in range(2):
                o_t = op.tile([128, BPOS], BF16, tag=f"o{h}", name=f"dd{b}{h}")
                nc.vector.tensor_copy(o_t[:], x_sb[h][:, bsl])
                nc.sync.dma_start(out_d[h * 128:(h + 1) * 128, bsl], o_t[:])
        return
    if stage == "proj":
        for b in range(NBATCH):
            emit_proj(b)
            k_t, q_t, vT_t = state.pop(b)
            dump(b, k_t, 0)
            dump(b, vT_t, 1)
        return
    if stage == "scores":
        emit_proj(0)
        for b in range(NBATCH):
            emit_scores(b)
            if b + 1 < NBATCH:
                emit_proj(b + 1)
            k_t, q_t, vT_t, aT_t = state.pop(b)
            dump(b, aT_t, 0)
        return
    emit_proj(0)
    for b in range(NBATCH):
        emit_scores(b)
        if b + 1 < NBATCH:
            emit_proj(b + 1)
        emit_tail(b)


def build():
    key = tuple(sorted(CFG.items()))
    if key in _CACHE:
        return _CACHE[key]
    nc = bacc.Bacc("TRN2", target_bir_lowering=False, debug=False, num_devices=8)
    xb = nc.dram_tensor("xb", [C, NPOS], BF16, kind="ExternalInput")
    wkT = nc.dram_tensor("wkT", [C, S], BF16, kind="ExternalInput")
    wqT = nc.dram_tensor("wqT", [C, S], BF16, kind="ExternalInput")
    wvT = nc.dram_tensor("wvT", [C, S], BF16, kind="ExternalInput")
    woT = nc.dram_tensor("woT", [S, C], BF16, kind="ExternalInput")
    bk2 = nc.dram_tensor("bk2", [S, 1], F32, kind="ExternalInput")
    bq2 = nc.dram_tensor("bq2", [S, 1], F32, kind="ExternalInput")
    catt = nc.dram_tensor("catt", [S, 1], F32, kind="ExternalInput")
    ident = nc.dram_tensor("ident", [128, 128], BF16, kind="ExternalInput")
    out_d = nc.dram_tensor("out", [C, NPOS], BF16, kind="ExternalOutput")
    from contextlib import ExitStack
    with tile.TileContext(nc) as tc, ExitStack() as ctx:
        _emit(nc, tc, (xb, wkT, wqT, wvT, woT, bk2, bq2, catt, ident, out_d),
              ctx)
    nc.compile()
    _CACHE[key] = nc
    return nc


def make_in_maps(x, wk, bk, wq, bq, wv, bv, wo, bo):
    bf = ml_dtypes.bfloat16
    x = np.asarray(x, np.float32)
    wk, wq, wv, wo = (np.asarray(a, np.float32) for a in (wk, wq, wv, wo))
    bk, bq, bv, bo = (np.asarray(a, np.float32) for a in (bk, bq, bv, bo))
    # hw-major: pos = hw*64 + d
    xr = x.reshape(B, C, D, HW).transpose(0, 1, 3, 2)          # [B,C,HW,D]
    xp = (xr + bo[None, :, None, None]).reshape(B, C, NPOS).astype(bf)
    com = {
        "wkT": np.ascontiguousarray(wk.T).astype(bf),
        "wqT": np.ascontiguousarray(wq.T).astype(bf),
        "wvT": np.ascontiguousarray(wv.T).astype(bf),
        "woT": np.ascontiguousarray(wo.T).astype(bf),
        "bk2": (bk - wk @ bo).reshape(S, 1),
        "bq2": (bq - wq @ bo).reshape(S, 1),
        "catt": (bv - wv @ bo).reshape(S, 1),
        "ident": np.eye(128, dtype=bf),
    }
    return [dict(com, xb=np.ascontiguousarray(xp[b])) for b in range(B)]


def run(x, wk, bk, wq, bq, wv, bv, wo, bo, **kw):
    nc = build()
    maps = make_in_maps(x, wk, bk, wq, bq, wv, bv, wo, bo)
    res = run_bass_kernel_spmd(nc, maps, core_ids=list(range(B)), **kw)
    out = np.stack([np.asarray(r["out"]).astype(np.float32)
                    for r in res.results])
    # [B, C, HW, D] -> [B, C, D, H, W]
    out = out.reshape(B, C, HW, D).transpose(0, 1, 3, 2)
    return np.ascontiguousarray(out).reshape(B, C, D, H, W), res


def kernel(x, wk, bk, wq, bq, wv, bv, wo, bo):
    out, _ = run(x, wk, bk, wq, bq, wv, bv, wo, bo)
    return out
